# revision 1
# baseline (speedup 1.0000x reference)
"""Local cross-attention Trainium2 kernel.

Strategy (8 NeuronCores, SPMD):
  - Host: queries split into 32 kd-tree leaves of 128 queries (split dim
    chosen per node to minimize the children's padded key-chunk total);
    each leaf gathers the EXACT union of its queries' neighborhoods
    (reference float32 mask semantics), padded to 128-multiples. Leaves
    sorted by chunk count; rank-group i (ranks 8i..8i+7) becomes slot i,
    one leaf per core, so per-slot loop bounds are uniform and minimal.
    The exact (chunk-key, query) 0/1 mask is shipped as an input tensor
    (m01) -- no on-device distance compute, no borderline corrections.
  - Device main loop, software-pipelined over all (slot, chunk) with the
    score matmuls running 2 chunks ahead of AV:
      scores: per chunk a fresh 2-bank PSUM tile; 8 heads via K=64
        half-masked matmuls (QM[g][c] zeroes the other head sharing the
        64-row contraction block), 2-way row-tiled (row offsets {0,64}
        -> different banks, safe concurrency);
      E = exp(s/sqrt(32)): ONE ACT op per chunk over all 8 heads
        [128, 1024] (ACT is the steady-state bottleneck at ~1.04us/chunk);
      mask multiply: DVE (5 head-blocks) + GPSIMD (3 blocks) vs m01;
      AV: ones-augmented V (M=33, 2-way col-tiled at partitions {0,64})
        accumulates output + softmax denominator; per-slot bank zeroed by
        one matmul so the 8 interleaved chains run start=False.
  - K/V projections stream through the pipeline (K groups 1+ and all
    V chunks use spare PSUM banks) so the first exp fires at ~7us.
  - Epilogue per slot: reciprocal of denominator rows from PSUM, PE
    broadcast over head dims, then partition-aligned multiplies split
    DVE/GPSIMD into a (parity, block) layout sb_OP; output projection
    uses a host-permuted WoP with zeroed dead rows (full-K, bank-safe),
    split so 3/4 of it runs before the last slot's epilogue.
  - Host gathers outputs back to original query order.

CoreSim cost-model time: 35.5us/core (baseline kernel: 111us).
"""
import sys, os
sys.path.insert(0, '/opt/trn_rl_repo')

import numpy as np
from contextlib import ExitStack

import ml_dtypes

F = 256           # feature dim
H = 8             # heads
D = 32            # head dim
R = 3.0
R2 = 9.0
NC = 8            # cores
P = 128
QS = 128          # queries per slot
NSLOT = 4         # slots per core (512 q / core)
NQ = NSLOT * QS

bf16 = ml_dtypes.bfloat16

# engine-assignment knobs (sweepable via env KCFG=vcopy,kcopy,rbs,y with
# values d(ve)/a(ct)/x(alternate))
_CFG = dict(zip(('vcopy', 'kcopy', 'rbs', 'y'),
                os.environ.get('KCFG', 'd,a,a,a').split(',')))


# ---------------------------------------------------------------- host staging
def _leaves(cc, mask):
    """Split all queries into 32 kd leaves of 128; at each node pick the
    median split (of the 3 dims) minimizing the children's padded key-chunk
    total (exact neighborhood unions)."""
    leaves = [np.arange(cc.shape[0])]
    while len(leaves) < NC * NSLOT:
        nxt = []
        for l in leaves:
            pts = cc[l]
            best = None
            for d in range(3):
                order = np.argsort(pts[:, d], kind='stable')
                half = len(l) // 2
                l0, l1 = l[order[:half]], l[order[half:]]
                w0 = int(mask[l0].any(0).sum())
                w1 = int(mask[l1].any(0).sum())
                key = -(-w0 // P) + -(-w1 // P)
                if best is None or key < best[0]:
                    best = (key, l0, l1)
            nxt.append(best[1])
            nxt.append(best[2])
        leaves = nxt
    return leaves


def _plan(cc, hc):
    """kd leaves + exact-union key windows + rank-grouped slot assignment."""
    # reference-semantics mask (same float32 expression as reference())
    mask = np.zeros((cc.shape[0], hc.shape[0]), bool)
    for q0 in range(0, cc.shape[0], 512):
        d2 = ((cc[q0:q0+512, None, :] - hc[None, :, :]) ** 2).sum(
            -1, dtype=np.float32)
        mask[q0:q0+512] = d2 <= R2
    leaves = _leaves(cc, mask)
    sels = [np.nonzero(mask[l].any(0))[0] for l in leaves]
    chunks = np.array([max(1, (len(s) + P - 1) // P) for s in sels])
    order = np.argsort(-chunks, kind='stable')
    cores = [[] for _ in range(NC)]
    KW = []
    for i in range(NSLOT):
        grp = order[i * NC:(i + 1) * NC]
        KW.append(int(chunks[grp[0]]))
        for c in range(NC):
            li = grp[c]
            cores[c].append((leaves[li], sels[li]))
    return cores, KW, mask


def _stage(inputs):
    cc = np.ascontiguousarray(np.asarray(inputs['current_coords'], np.float32))
    hc = np.ascontiguousarray(np.asarray(inputs['historical_coords'], np.float32))
    cf = np.asarray(inputs['current_feats'], np.float32)
    hf = np.asarray(inputs['historical_feats'], np.float32)

    cores, KW, mask = _plan(cc, hc)
    NKP = sum(KW) * P          # padded key-instances per core

    # weights (shared across cores)
    WqT = np.ascontiguousarray(np.asarray(inputs['Wq'], np.float32).T).astype(bf16)
    WkT = np.ascontiguousarray(np.asarray(inputs['Wk'], np.float32).T).astype(bf16)
    WvT = np.ascontiguousarray(np.asarray(inputs['Wv'], np.float32).T).astype(bf16)
    WoT = np.ascontiguousarray(np.asarray(inputs['Wo'], np.float32).T).astype(bf16)
    bq = np.asarray(inputs['bq'], np.float32)
    bk = np.asarray(inputs['bk'], np.float32)
    bv = np.asarray(inputs['bv'], np.float32)
    bo = np.asarray(inputs['bo'], np.float32)
    bqk = np.stack([bq[:P], bq[P:], bk[:P], bk[P:]], 1)        # [128, 4]
    boT = np.stack([bo[:P], bo[P:]], 1)                        # [128, 2]
    bv_row = bv[None, :].astype(bf16)                          # [1, 256]
    vbias = bool(np.any(bv != 0.0))
    kbias = bool(np.any(bk != 0.0))
    obias = bool(np.any(bo != 0.0))
    # Wo rows permuted to the epilogue's (parity, block) AV layout:
    # WoP[64*(h%2)+d, h//2, e] = Wo[e, 32*h+d]; dead rows zero.
    WoP = np.zeros((P, 4, F), np.float32)
    for h in range(H):
        rho, b = h % 2, h // 2
        WoP[64*rho:64*rho+D, b, :] = WoT[32*h:32*h+D, :].astype(np.float32)
    WoP = np.ascontiguousarray(WoP.reshape(P, 4*F)).astype(bf16)

    in_maps = []
    qmaps = []          # original query indices in slot order, per core
    for c in range(NC):
        subs = cores[c]
        qsel = np.concatenate([s[0] for s in subs])
        qmaps.append(qsel)
        # key-instance arrays + exact per-(chunk, query) mask from the host
        kfeat = np.zeros((NKP, F), np.float32)
        m01h = np.zeros((P, NKP // P, QS), bf16)
        off = 0
        for i, (qs, sel) in enumerate(subs):
            kfeat[off:off + len(sel)] = hf[sel]
            sub = mask[np.ix_(qs, sel)].T.astype(bf16)   # [nsel, 128]
            for cix in range(KW[i]):
                lo = cix * P
                hi = min(len(sel), lo + P)
                if hi > lo:
                    m01h[0:hi - lo, off // P + cix, :] = sub[lo:hi]
            off += KW[i] * P
        in_maps.append({
            'zeros': np.zeros((P, 2048), bf16),
            'histTf': np.ascontiguousarray(kfeat.T).astype(bf16),
            'm01': np.ascontiguousarray(m01h.reshape(P, NKP)),
            'curT': np.ascontiguousarray(cf[qsel].T).astype(bf16),
            'wqT': WqT, 'wkT': WkT, 'wvT': WvT, 'woP': WoP,
            'bqk': bqk, 'boT': boT, 'bv_row': bv_row,
        })
    return in_maps, qmaps, KW, NKP, (vbias, kbias, obias)


# ---------------------------------------------------------------- bass kernel
def _build(KW, NKP, vbias=(False, False), reps=1):
    import concourse.bass as bass
    import concourse.bacc as bacc
    import concourse.tile as tile
    from concourse import mybir

    f32 = mybir.dt.float32
    b16 = mybir.dt.bfloat16
    NCH = NKP // P
    ISCALE = 1.0 / np.sqrt(D)

    nc = bacc.Bacc("TRN2", target_bir_lowering=False, debug=False,
                   enable_asserts=False, num_devices=NC)

    t_zeros = nc.dram_tensor('zeros', [P, 2048], b16, kind='ExternalInput')
    t_histTf = nc.dram_tensor('histTf', [F, NKP], b16, kind='ExternalInput')
    t_m01 = nc.dram_tensor('m01', [P, NKP], b16, kind='ExternalInput')
    t_curT = nc.dram_tensor('curT', [F, NQ], b16, kind='ExternalInput')
    t_wqT = nc.dram_tensor('wqT', [F, F], b16, kind='ExternalInput')
    t_wkT = nc.dram_tensor('wkT', [F, F], b16, kind='ExternalInput')
    t_wvT = nc.dram_tensor('wvT', [F, F], b16, kind='ExternalInput')
    t_woP = nc.dram_tensor('woP', [P, 4 * F], b16, kind='ExternalInput')
    t_bqk = nc.dram_tensor('bqk', [P, 4], f32, kind='ExternalInput')
    t_boT = nc.dram_tensor('boT', [P, 2], f32, kind='ExternalInput')
    t_bv = nc.dram_tensor('bv_row', [1, F], b16, kind='ExternalInput')
    t_yT = nc.dram_tensor('yT', [F, NQ], f32, kind='ExternalOutput')

    base = np.cumsum([0] + KW)          # chunk base per slot

    with tile.TileContext(nc) as tc, ExitStack() as ctx:
        sing = ctx.enter_context(tc.tile_pool(name='sing', bufs=1))
        epool = ctx.enter_context(tc.tile_pool(name="epool", bufs=3))
        opool = ctx.enter_context(tc.tile_pool(name='opool', bufs=2))
        ps_sc = ctx.enter_context(tc.tile_pool(name='ps_sc', bufs=2, space='PSUM'))
        ps_d2 = ctx.enter_context(tc.tile_pool(name='ps_d2', bufs=1, space='PSUM'))
        ps_av = ctx.enter_context(tc.tile_pool(name='ps_av', bufs=2, space='PSUM'))
        ps_rb = ctx.enter_context(tc.tile_pool(name='ps_rb', bufs=1, space='PSUM'))

        for _rep in range(reps):
            _emit_once(nc, tc, mybir, KW, NKP, base, NCH, ISCALE, vbias,
                       sing, epool, opool, ps_sc, ps_d2, ps_av, ps_rb,
                       t_histTf, t_m01, t_curT, t_wqT, t_wkT, t_wvT,
                       t_woP, t_bqk, t_boT, t_bv, t_yT, t_zeros, f32, b16)

    nc.compile()
    return nc


def _emit_once(nc, tc, mybir, KW, NKP, base, NCH, ISCALE, vbias,
               sing, epool, opool, ps_sc, ps_d2, ps_av, ps_rb,
               t_histTf, t_m01, t_curT, t_wqT, t_wkT, t_wvT,
               t_woP, t_bqk, t_boT, t_bv, t_yT, t_zeros, f32, b16):
    vbias, kbias, obias = vbias
    Exp = mybir.ActivationFunctionType.Exp
    Ident = mybir.ActivationFunctionType.Identity

    # ---------------- load inputs (sync + gpsimd DMA queues, urgency order;
    # nothing on the scalar queue -- ACT-issued DMAs delay ACT compute)
    sb_hist = [sing.tile([P, NKP], b16, tag=f'hist{g}', name=f'hist{g}')
               for g in range(2)]
    sb_curT = [sing.tile([P, NQ], b16, tag=f'curT{g}', name=f'curT{g}')
               for g in range(2)]
    sb_m01 = sing.tile([P, NKP], b16)
    sb_w = {}
    for nm, t in (('q', t_wqT), ('k', t_wkT), ('v', t_wvT)):
        sb_w[nm] = [sing.tile([P, F], b16, tag=f'w{nm}{g}', name=f'w{nm}{g}')
                    for g in range(2)]
    sb_woP = sing.tile([P, 4, F], b16)
    sb_bqk = sing.tile([P, 4], f32)
    sb_boT = sing.tile([P, 2], f32)
    sb_bv = sing.tile([1, F], b16)
    hhalf = (NKP // 2 // 64) * 64 or NKP
    nc.sync.dma_start(out=sb_w['q'][0], in_=t_wqT.ap()[0:P, :])
    nc.gpsimd.dma_start(out=sb_w['q'][1], in_=t_wqT.ap()[P:2*P, :])
    nc.sync.dma_start(out=sb_curT[0], in_=t_curT.ap()[0:P, :])
    nc.gpsimd.dma_start(out=sb_curT[1], in_=t_curT.ap()[P:2*P, :])
    nc.sync.dma_start(out=sb_bqk, in_=t_bqk.ap())
    nc.gpsimd.dma_start(out=sb_w['k'][1], in_=t_wkT.ap()[P:2*P, :])
    nc.sync.dma_start(out=sb_w['k'][0], in_=t_wkT.ap()[0:P, :])
    nc.sync.dma_start(out=sb_hist[0], in_=t_histTf.ap()[0:P, :])
    nc.gpsimd.dma_start(out=sb_hist[1], in_=t_histTf.ap()[P:2*P, :])
    sb_QM = [[sing.tile([P, NQ], b16, tag=f'QM{g}{c}', name=f'QM{g}{c}')
              for c in range(2)] for g in range(2)]
    for g in range(2):
        for c in range(2):
            (nc.sync if c == 0 else nc.gpsimd).dma_start(
                out=sb_QM[g][c], in_=t_zeros.ap()[:, 0:NQ])
    nc.sync.dma_start(out=sb_w['v'][0], in_=t_wvT.ap()[0:P, :])
    nc.gpsimd.dma_start(out=sb_w['v'][1], in_=t_wvT.ap()[P:2*P, :])
    nc.sync.dma_start(out=sb_m01[:, :hhalf], in_=t_m01.ap()[:, :hhalf])
    nc.gpsimd.dma_start(out=sb_m01[:, hhalf:], in_=t_m01.ap()[:, hhalf:])
    nc.gpsimd.dma_start(out=sb_bv, in_=t_bv.ap())
    nc.gpsimd.dma_start(out=sb_woP, in_=t_woP.ap())
    nc.gpsimd.dma_start(out=sb_boT, in_=t_boT.ap())
    sb_one = sing.tile([1, P], b16)
    nc.vector.memset(sb_one, 1.0)
    sb_zero = sing.tile([1, 512], b16)
    nc.gpsimd.dma_start(out=sb_zero, in_=t_zeros.ap()[0:1, 0:512])
    # tiny dummy exp so the ACT table load runs during the DMA wait instead
    # of blocking the first real activation
    sb_dummy = sing.tile([1, 8], b16)
    nc.scalar.activation(sb_dummy, sb_one[0:1, 0:8],
                         mybir.ActivationFunctionType.Exp)

    # ---------------- PSUM layout
    vps = ps_d2.tile([P, 512], f32, tag='vps', name='vps')
    rb = ps_rb.tile([P, 512], f32, tag='rb', name='rb')
    nc.vector.memset(rb[D:64, :], 0.0)
    nc.vector.memset(rb[64 + D:128, :], 0.0)

    def proj_ps():
        return ps_sc.tile([P, 1024], f32, tag='sc', name='ps')

    # Q^T then masked-Q tiles: QM[g][c] has, in each 64-row block, only the
    # c-th 32-row half live (head a uses block a//2 of QM[g][a%2]; the other
    # head sharing that block contributes zero). Copies split DVE/ACT.
    sb_QT = [sing.tile([P, NQ], b16, tag=f'QT{g}', name=f'QT{g}')
             for g in range(2)]
    for g in range(2):
        ps = proj_ps()
        for j in range(2):
            nc.tensor.matmul(ps[:, :NQ], sb_w['q'][j][:, g * P:(g + 1) * P],
                             sb_curT[j], start=(j == 0), stop=(j == 1))
        nc.scalar.activation(sb_QT[g], ps[:, :NQ], Ident,
                             bias=sb_bqk[:, g:g + 1])
    for g in range(2):
        for c in range(2):
            for b in range(2):
                r = 64 * b + 32 * c
                nc.vector.tensor_copy(sb_QM[g][c][r:r + 32, :],
                                      sb_QT[g][r:r + 32, :])

    # K^T [f, k] (bf16): group 0 in the prologue (sc-pool psum); later
    # groups are emitted inside the pipeline on the rb bank so exp(0) does
    # not queue behind their ACT copies.
    sb_KT = [sing.tile([P, NKP], b16, tag=f'KT{g}', name=f'KT{g}') for g in range(2)]

    def emit_K(j4, lazy):
        w = min(4, NCH - j4) * P
        for g in range(2):
            ps = rb[:, 0:w] if lazy else proj_ps()[:, 0:w]
            for j in range(2):
                nc.tensor.matmul(ps, sb_w['k'][j][:, g * P:(g + 1) * P],
                                 sb_hist[j][:, j4 * P:j4 * P + w],
                                 start=(j == 0), stop=(j == 1),
                                 skip_group_check=lazy)
            if kbias or not lazy:
                nc.scalar.activation(sb_KT[g][:, j4 * P:j4 * P + w], ps,
                                     Ident, bias=sb_bqk[:, 2 + g:3 + g])
            else:
                nc.vector.tensor_copy(sb_KT[g][:, j4 * P:j4 * P + w], ps)

    emit_K(0, lazy=False)
    # V [k, h*33+d] (bf16) with ones column per head (preset once); the
    # per-chunk projection is emitted inside the pipeline (psum = rb bank)
    sb_V = sing.tile([P, NCH, H * 33], b16)
    nc.vector.memset(sb_V.rearrange('p c (h x) -> p c h x', h=H)[:, :, :, D:D + 1],
                     1.0)

    def emit_V(c):
        ps = vps[:, 0:F]
        for g in range(2):
            nc.tensor.matmul(ps, sb_hist[g][:, c * P:(c + 1) * P],
                             sb_w['v'][g], start=(g == 0),
                             stop=(g == 1 and not vbias),
                             skip_group_check=True)
        if vbias:
            nc.tensor.matmul(ps, sb_one[0:1, :P], sb_bv,
                             start=False, stop=True, skip_group_check=True)
        vv = sb_V[:, c, :].rearrange('p (h x) -> p h x', h=H)
        pv = ps.rearrange('p (h x) -> p h x', h=H)
        if _CFG['vcopy'] == 'd' or (_CFG['vcopy'] == 'x' and c % 2 == 0):
            nc.vector.tensor_copy(vv[:, :, 0:D], pv)
        else:
            nc.scalar.copy(vv[:, :, 0:D], pv)

    # ---------------- main loop: software-pipelined over all (slot, chunk)
    # PE stream per step t: [d2 group?] S(t); AV(t-2) — scores run 2 chunks
    # ahead of AV so the PE never stalls on the exp->mask chain.
    # normalized AV in (parity, block) layout: rows 64*(h%2)+d, block h//2
    sb_OP = sing.tile([P, 4, NQ], b16)
    nc.sync.dma_start(out=sb_OP, in_=t_zeros.ap())
    av_tiles = {}

    chunks = [(s, j) for s in range(len(KW)) for j in range(KW[s])]
    n = len(chunks)
    sc_tiles = {}  # t -> per-chunk score psum tile
    e_tiles = {}   # t -> e tile
    pending_epi = []

    def emit_S(t):
        s, j = chunks[t]
        qsl = slice(s * QS, (s + 1) * QS)
        kc = (base[s] + j) * P
        ksl = slice(kc, kc + P)
        # scores: 8 heads, K=64 half-masked, 2-way row-tiled: head (g, a)
        # contracts KT rows 64*(a//2)..+64 against QM[g][a%2]; row tiles at
        # {0, 64} write different banks of this chunk's 2-bank tile.
        sc = ps_sc.tile([P, 1024], f32, tag='sc', name='sc')
        scv = sc.rearrange('p (b g c q) -> p b g c q', b=2, g=2, c=2)
        for g in range(2):
            for a in (0, 2, 1, 3):
                b, c = a // 2, a % 2
                nc.tensor.matmul(
                    scv[:, b, g, c, :],
                    sb_KT[g][64 * b:64 * b + 64, ksl],
                    sb_QM[g][c][64 * b:64 * b + 64, qsl],
                    start=True, stop=True,
                    tile_position=(64 * b, 0))
        sc_tiles[t] = sc

    def emit_EM(t):
        s, j = chunks[t]
        sc = sc_tiles.pop(t)
        e = epool.tile([P, 2, 2, 2, P], b16, tag='e', name='e')
        nc.scalar.activation(e, sc, Exp, scale=ISCALE)
        ef = e.rearrange('p b g c q -> p (b g c) q')
        kc = (base[s] + j) * P
        msl = sb_m01[:, None, kc:kc + P]
        nc.vector.tensor_tensor(ef[:, 0:5, :], ef[:, 0:5, :],
                                msl.to_broadcast([P, 5, P]),
                                mybir.AluOpType.mult)
        nc.gpsimd.tensor_tensor(ef[:, 5:8, :], ef[:, 5:8, :],
                                msl.to_broadcast([P, 3, P]),
                                mybir.AluOpType.mult)
        e_tiles[t] = e

    def emit_AV(t):
        s, j = chunks[t]
        if j == 0:
            # zero the whole av bank and set every has_written bit so the 8
            # interleaved per-head accumulation chains can run start=False
            # (a per-head start=True would re-mark the whole 2KB zero region
            # and turn other heads' accumulations into overwrites).
            av = av_tiles[s] = ps_av.tile([P, 512], f32, tag='av', name='av')
            nc.tensor.matmul(av, sb_zero[0:1, 0:P], sb_zero[0:1, :],
                             start=True, stop=False, skip_group_check=True)
        av = av_tiles[s]
        e = e_tiles.pop(t)
        nkc = KW[s]
        for h in range(H):
            g, a = divmod(h, 4)
            po = 64 * (h % 2)
            fo = 128 * (h // 2)
            nc.tensor.matmul(
                av[po:po + 33, fo:fo + QS],
                sb_V[:, base[s] + j, 33 * h:33 * h + 33],
                e[:, a // 2, g, a % 2, :],
                start=False, stop=(j == nkc - 1 and h == H - 1),
                skip_group_check=True,
                tile_position=(0, po))
        if j == nkc - 1:
            pending_epi.append([s, 0])

    def emit_epilogue(s):
        qsl = slice(s * QS, (s + 1) * QS)
        av = av_tiles.pop(s)
        av_sb = opool.tile([P, 512], b16, tag='avsb', name='av_sb')
        nc.scalar.copy(av_sb, av)
        # reciprocal of the denominator rows straight from PSUM (runs in
        # parallel with the av_sb copy above), to partition 0 so the PE
        # broadcast matmul has lhsT/rhs at the same partition.
        rec = opool.tile([1, 1024], b16, tag='rec', name='rec')
        with nc.allow_low_precision(reason='softmax denom reciprocal in bf16; '
                                    'rel tol 2e-2 dominates'):
            nc.vector.reciprocal(rec[0:1, 0:512], av[32:33, :])
            nc.vector.reciprocal(rec[0:1, 512:1024], av[96:97, :])
        for h in range(H):
            po = 64 * (h % 2)
            fo = 128 * (h // 2)
            nc.tensor.matmul(rb[po:po + D, fo:fo + QS],
                             sb_one[0:1, 0:D],
                             rec[0:1, 512 * (h % 2) + fo:512 * (h % 2) + fo + QS],
                             start=True, stop=True,
                             tile_position=(0, po))
        rbs = opool.tile([P, 512], b16, tag='rbs', name='rbs')
        if _CFG['rbs'] == 'a':
            nc.scalar.copy(rbs, rb)
        else:
            nc.vector.tensor_copy(rbs, rb)
        for rho in range(2):
            eng = nc.vector if rho == 0 else nc.gpsimd
            eng.tensor_tensor(
                sb_OP[64 * rho:64 * rho + D, :, qsl],
                av_sb[64 * rho:64 * rho + D, :].rearrange(
                    'p (b q) -> p b q', b=4),
                rbs[64 * rho:64 * rho + D, :].rearrange(
                    'p (b q) -> p b q', b=4),
                mybir.AluOpType.mult)

    def emit_oproj(c0, c1):
        # output projection for query columns [c0:c1) (K=128 per block;
        # dead rows of WoP are zero)
        w = c1 - c0
        for g2 in range(2):
            ps = proj_ps()
            for b in range(4):
                nc.tensor.matmul(ps[:, :w],
                                 sb_woP[:, b, g2 * P:(g2 + 1) * P],
                                 sb_OP[:, b, c0:c1],
                                 start=(b == 0), stop=(b == 3))
            y = opool.tile([P, NQ], f32, tag='y', name='y')
            if obias or _CFG['y'] == 'a':
                nc.scalar.activation(y[:, :w], ps[:, :w], Ident,
                                     bias=sb_boT[:, g2:g2 + 1])
            else:
                nc.vector.tensor_copy(y[:, :w], ps[:, :w])
            (nc.sync if g2 == 0 else nc.gpsimd).dma_start(
                out=t_yT.ap()[g2 * P:(g2 + 1) * P, c0:c1], in_=y[:, :w])

    emit_V(0)
    emit_V(1)
    done_epi = set()
    for t in range(n + 2):
        # age pending epilogues; emit once the slot's last AV is 2 steps old
        for ent in list(pending_epi):
            ent[1] += 1
            if ent[1] >= 3 or t >= n:
                emit_epilogue(ent[0])
                done_epi.add(ent[0])
                pending_epi.remove(ent)
        if t < n:
            emit_S(t)
        if t % 4 == 0 and t + 4 < NCH * 1 and (t + 4) % 4 == 0:
            emit_K(t + 4, lazy=True)
        if t + 2 < n:
            emit_V(t + 2)
        if 1 <= t <= n:
            emit_EM(t - 1)
        if t >= 2:
            emit_AV(t - 2)
        if t == n and all(s in done_epi for s in range(NSLOT - 1)):
            emit_oproj(0, (NSLOT - 1) * QS)
    for ent in pending_epi:
        emit_epilogue(ent[0])
        done_epi.add(ent[0])
    if all(s in done_epi for s in range(NSLOT - 1)) and len(done_epi) == NSLOT:
        emit_oproj((NSLOT - 1) * QS, NQ)
    else:
        emit_oproj(0, NQ)


_CACHE = {}


def kernel(**inputs):
    from concourse import bass_utils

    in_maps, qmaps, KW, NKP, vbias = _stage(inputs)
    key = (tuple(KW), vbias)
    if key not in _CACHE:
        _CACHE[key] = _build(KW, NKP, vbias)
    nc = _CACHE[key]
    res = bass_utils.run_bass_kernel_spmd(nc, in_maps, core_ids=list(range(NC)))
    N = inputs['current_feats'].shape[0]
    out = np.zeros((N, F), np.float32)
    for c in range(NC):
        out[qmaps[c]] = res.results[c]['yT'].T
    return out


if __name__ == '__main__':
    pass

